# revision 1
# baseline (speedup 1.0000x reference)
"""Trainium2 Bass kernel for a dense transformer block (nn_Block_86595130622141).

Reference computation (B=1, T=4096, C=1024, H=16 heads, hd=64, FFN=4C):
    h   = LN(x, g1, be1)
    q,k,v = per-head projections of h         (Wq/Wk/Wv: [H, C, hd])
    attn  = causal softmax(q k^T / sqrt(C)) v  (per head)
    x2  = x + concat(attn) @ Wp + bp
    out = x2 + relu(LN(x2, g2, be2) @ W1 + b1) @ W2 + b2

Sharding over 8 NeuronCores:
  - attention (incl. q/k/v projections) is tensor-parallel over heads:
    core j owns heads {2j, 2j+1} over the full sequence,
  - projection/LN2/FFN/residuals are sequence-parallel: core j owns rows
    [512j, 512j+512),
  - two bf16 AllToAlls (512 KiB/core each, one per local head) re-shard the
    attention output from head-parallel to sequence-parallel; the first
    overlaps with the second head's attention compute.
LN gammas/betas are algebraically folded into the adjacent projection
weights/biases on the host (pure weight preprocessing), so the device only
computes the core normalization (x - mu) * rstd.
All matmuls run in bf16 with fp32 PSUM accumulation; LN statistics,
softmax sums and both residual paths stay in fp32 (residual and bias adds
are accumulated straight into PSUM via fp32 identity/ones matmuls).
"""

import numpy as np
import ml_dtypes

import concourse.bass as bass
import concourse.mybir as mybir
import concourse.tile as tile
from concourse import bacc
from concourse.bass_utils import run_bass_kernel_spmd
from concourse.masks import make_identity

F32 = mybir.dt.float32
BF16 = mybir.dt.bfloat16
AF = mybir.ActivationFunctionType
ALU = mybir.AluOpType

N_CORES = 8
T = 4096
C = 1024
H = 16
HD = 64
FF = 4096
LN_EPS = 1e-5
ROWS = T // N_CORES          # 512 rows per core (sequence shard)
HPC = H // N_CORES           # 2 heads per core
D2 = HPC * HD                # 128: stacked head dim per core
NCT = C // 128               # 8 c-tiles
NTB = T // 512               # 8 t-blocks of 512
NFB = FF // 128              # 32 f-tiles
SCALE = float(C) ** -0.5


def _bf16(a):
    return np.ascontiguousarray(np.asarray(a, dtype=np.float32)).astype(ml_dtypes.bfloat16)


def _f32(a):
    return np.ascontiguousarray(np.asarray(a, dtype=np.float32))


def build_program():
    nc = bacc.Bacc("TRN2", target_bir_lowering=False, debug=False,
                   num_devices=N_CORES)

    dt = nc.dram_tensor
    d = {
        "x": dt("x", [T, C], F32, kind="ExternalInput").ap(),
        "xr": dt("xr", [ROWS, C], F32, kind="ExternalInput").ap(),
        "wq": dt("wq", [NCT, 128, D2], BF16, kind="ExternalInput").ap(),
        "wk": dt("wk", [NCT, 128, D2], BF16, kind="ExternalInput").ap(),
        "wv": dt("wv", [NCT, 128, D2], BF16, kind="ExternalInput").ap(),
        "bq": dt("bq", [D2], F32, kind="ExternalInput").ap(),
        "bk": dt("bk", [D2], F32, kind="ExternalInput").ap(),
        "bv": dt("bv", [D2], F32, kind="ExternalInput").ap(),
        "wp": dt("wp", [NCT, 128, C], BF16, kind="ExternalInput").ap(),
        "bp": dt("bp", [C], F32, kind="ExternalInput").ap(),
        "w1": dt("w1", [NFB, 128, NCT, 128], BF16, kind="ExternalInput").ap(),
        "b1": dt("b1", [128, NFB], F32, kind="ExternalInput").ap(),
        "w2": dt("w2", [NFB, 128, C], BF16, kind="ExternalInput").ap(),
        "b2": dt("b2", [C], F32, kind="ExternalInput").ap(),
        "cmask": dt("cmask", [4, 128, 512], BF16, kind="ExternalInput").ap(),
        "out": dt("out", [ROWS, C], F32, kind="ExternalOutput").ap(),
        "a2ai0": dt("a2a_in0", [N_CORES, 64, 512], BF16).ap(),
        "a2ao0": dt("a2a_out0", [N_CORES, 64, 512], BF16).ap(),
        "a2ai1": dt("a2a_in1", [N_CORES, 64, 512], BF16).ap(),
        "a2ao1": dt("a2a_out1", [N_CORES, 64, 512], BF16).ap(),
    }

    with tile.TileContext(nc) as tc:
        _build(nc, tc, d)
    nc.compile()
    return nc


def _build(nc, tc, d):
    from contextlib import ExitStack

    stack = ExitStack()
    with stack:
        const = stack.enter_context(tc.tile_pool(name="const", bufs=1))

        # ---------- constants ----------
        ident = const.tile([128, 128], BF16)
        make_identity(nc, ident)
        ident_f = const.tile([128, 128], F32)
        make_identity(nc, ident_f)
        eps_t = const.tile([128, 1], F32)
        nc.vector.memset(eps_t, LN_EPS)
        ones1 = const.tile([1, 128], F32)
        nc.vector.memset(ones1, 1.0)
        ones65 = const.tile([65, 128], F32)
        nc.vector.memset(ones65, 1.0)

        cm_sb = const.tile([128, 4, 512], BF16)
        for r in range(4):
            nc.sync.dma_start(out=cm_sb[:, r, :], in_=d["cmask"][r])

        bq_sb = const.tile([128, 1], F32)
        nc.sync.dma_start(out=bq_sb, in_=d["bq"][:])
        bk_sb = const.tile([128, 1], F32)
        nc.sync.dma_start(out=bk_sb, in_=d["bk"][:])
        bp_row = const.tile([1, C], F32)
        nc.sync.dma_start(out=bp_row, in_=d["bp"][:])
        b2_row = const.tile([1, C], F32)
        nc.sync.dma_start(out=b2_row, in_=d["b2"][:])
        b1_sb = const.tile([128, NFB], F32)
        nc.sync.dma_start(out=b1_sb, in_=d["b1"])

        # bv broadcast across partitions: [t-partition, d2] via K=1 matmul
        bv_bc = const.tile([128, D2], F32)
        with tc.tile_pool(name="bcast_ps", bufs=1, space="PSUM") as bcast_ps:
            bv_row = const.tile([1, D2], F32)
            nc.sync.dma_start(out=bv_row, in_=d["bv"][:])
            ps_bv = bcast_ps.tile([128, D2], F32, tag="bc")
            nc.tensor.matmul(ps_bv, lhsT=ones1, rhs=bv_row, start=True, stop=True)
            nc.vector.tensor_copy(out=bv_bc, in_=ps_bv)

        # ---------- attention-phase persistent tensors ----------
        attn_stack = stack.enter_context(ExitStack())
        attn_pool = attn_stack.enter_context(tc.tile_pool(name="attn_p", bufs=1))
        qT = [attn_pool.tile([128, 512], BF16, tag=f"qT{b}", name=f"qT{b}")
              for b in range(NTB)]
        kT = [attn_pool.tile([128, 512], BF16, tag=f"kT{b}", name=f"kT{b}")
              for b in range(NTB)]
        # v with a trailing ones column per head (softmax sums land on PSUM
        # partition 64 = 32-aligned): [tk_r, tk_tile_in_block, head, 64+1]
        vv = [attn_pool.tile([128, 4, HPC, 65], BF16, tag=f"v{b}", name=f"v{b}")
              for b in range(NTB)]
        for b in range(NTB):
            nc.vector.memset(vv[b][:, :, :, 64:65], 1.0)
        attnT = [attn_pool.tile([64, HPC, 512], BF16, tag=f"aT{b}", name=f"aT{b}")
                 for b in range(NTB)]

        # ---------- LN1 + transpose + QKV, interleaved per t-block ----------
        with (
            tc.tile_pool(name="h1T_p", bufs=1) as h1T_p,
            tc.tile_pool(name="ln_x", bufs=4) as ln_x,
            tc.tile_pool(name="ln_tmp", bufs=4) as ln_tmp,
            tc.tile_pool(name="tr_ps", bufs=2, space="PSUM") as tr_ps,
            tc.tile_pool(name="qkv_ps", bufs=2, space="PSUM") as qkv_ps,
            tc.tile_pool(name="wqkv", bufs=1) as wqkv,
        ):
            wq_sb = wqkv.tile([128, NCT, D2], BF16)
            wk_sb = wqkv.tile([128, NCT, D2], BF16)
            wv_sb = wqkv.tile([128, NCT, D2], BF16)
            for ci in range(NCT):
                nc.sync.dma_start(out=wq_sb[:, ci, :], in_=d["wq"][ci])
                nc.sync.dma_start(out=wk_sb[:, ci, :], in_=d["wk"][ci])
                nc.sync.dma_start(out=wv_sb[:, ci, :], in_=d["wv"][ci])

            h1T = [h1T_p.tile([128, NCT, 512], BF16, tag=f"h1T{b}", name=f"h1T{b}")
                   for b in range(NTB)]

            for tb in range(NTB):
                for tsub in range(4):
                    ti = 4 * tb + tsub
                    x_t = ln_x.tile([128, C], F32)
                    nc.sync.dma_start(out=x_t, in_=d["x"][128 * ti:128 * ti + 128])
                    stats = ln_tmp.tile([128, 2, 6], F32, tag="stats")
                    xr2 = x_t.rearrange("p (s d) -> p s d", s=2)
                    for sg in range(2):
                        nc.vector.bn_stats(out=stats[:, sg, :], in_=xr2[:, sg, :])
                    mv = ln_tmp.tile([128, 2], F32, tag="mv")
                    nc.vector.bn_aggr(out=mv, in_=stats)
                    rstd = ln_tmp.tile([128, 1], F32, tag="rstd")
                    nc.scalar.activation(out=rstd, in_=mv[:, 1:2], func=AF.Sqrt,
                                         bias=eps_t, scale=1.0)
                    nc.vector.reciprocal(out=rstd, in_=rstd)
                    h1c = ln_tmp.tile([128, C], BF16, tag="h1c")
                    nc.vector.tensor_scalar(out=h1c, in0=x_t, scalar1=mv[:, 0:1],
                                            scalar2=rstd, op0=ALU.subtract,
                                            op1=ALU.mult)
                    for ci in range(NCT):
                        ps_t = tr_ps.tile([128, 128], BF16, tag="tr")
                        nc.tensor.transpose(ps_t, h1c[:, 128 * ci:128 * ci + 128], ident)
                        nc.vector.tensor_copy(
                            out=h1T[tb][:, ci, 128 * tsub:128 * tsub + 128], in_=ps_t)

                # q^T / k^T for this t-block: out[d2=128, t=512]
                ps_q = qkv_ps.tile([128, 512], F32, tag="q")
                ps_k = qkv_ps.tile([128, 512], F32, tag="k")
                for ci in range(NCT):
                    nc.tensor.matmul(ps_q, lhsT=wq_sb[:, ci, :], rhs=h1T[tb][:, ci, :],
                                     start=(ci == 0), stop=(ci == NCT - 1))
                for ci in range(NCT):
                    nc.tensor.matmul(ps_k, lhsT=wk_sb[:, ci, :], rhs=h1T[tb][:, ci, :],
                                     start=(ci == 0), stop=(ci == NCT - 1))
                nc.vector.tensor_scalar_add(out=qT[tb], in0=ps_q, scalar1=bq_sb)
                nc.vector.tensor_scalar_add(out=kT[tb], in0=ps_k, scalar1=bk_sb)
                # v natural: out[t=128, d2], lhsT = h1T tile [c_tile, t_tile]
                for tsub in range(4):
                    ps_v = qkv_ps.tile([128, D2], F32, tag="v")
                    for ci in range(NCT):
                        nc.tensor.matmul(
                            ps_v,
                            lhsT=h1T[tb][:, ci, 128 * tsub:128 * tsub + 128],
                            rhs=wv_sb[:, ci, :],
                            start=(ci == 0), stop=(ci == NCT - 1))
                    nc.vector.tensor_tensor(
                        out=vv[tb][:, tsub, :, 0:64],
                        in0=ps_v.rearrange("p (h d) -> p h d", h=HPC),
                        in1=bv_bc.rearrange("p (h d) -> p h d", h=HPC),
                        op=ALU.add)

        # ---------- attention: head-outer, two interleaved t-block streams,
        # split AllToAll per head (the first overlaps head-1 compute) --------
        a2a_io = [(d["a2ai0"], d["a2ao0"]), (d["a2ai1"], d["a2ao1"])]
        with (
            tc.tile_pool(name="at_ps_s", bufs=4, space="PSUM") as ps_s_p,
            tc.tile_pool(name="at_ps_a", bufs=3, space="PSUM") as ps_a_p,
            tc.tile_pool(name="at_ps_b", bufs=1, space="PSUM") as ps_b_p,
            tc.tile_pool(name="at_w", bufs=8) as w_p,
            tc.tile_pool(name="at_sm", bufs=4) as sm_p,
        ):
            def at_scores(h, tb, tkt):
                hs = 64 * h
                tkb, tks = tkt // 4, tkt % 4
                ps_s = ps_s_p.tile([128, 512], F32, tag="ps", name=f"s{h}{tb}{tkt}")
                nc.tensor.matmul(
                    ps_s,
                    lhsT=kT[tkb][hs:hs + 64, 128 * tks:128 * tks + 128],
                    rhs=qT[tb][hs:hs + 64, :],
                    start=True, stop=True)
                w_t = w_p.tile([128, 512], BF16, tag="w", name=f"w{h}{tb}{tkt}")
                nc.scalar.activation(out=w_t, in_=ps_s, func=AF.Exp, scale=SCALE)
                if tkt >= 4 * tb:
                    nc.vector.tensor_tensor(
                        out=w_t, in0=w_t, in1=cm_sb[:, tkt - 4 * tb, :],
                        op=ALU.mult)
                return w_t

            def at_av(h, tb, tkt, ps_a, w_t):
                tkb, tks = tkt // 4, tkt % 4
                ntk = 4 * (tb + 1)
                nc.tensor.matmul(ps_a, lhsT=vv[tkb][:, tks, h, :], rhs=w_t,
                                 start=(tkt == 0), stop=(tkt == ntk - 1))

            def at_finish(h, tb, ps_a):
                recip = sm_p.tile([65, 512], F32, tag="recip", name=f"r{h}{tb}")
                nc.vector.reciprocal(out=recip[64:65, :], in_=ps_a[64:65, :])
                ps_bc = ps_b_p.tile([64, 512], F32, tag="pb", name=f"pb{h}{tb}")
                nc.tensor.matmul(ps_bc, lhsT=ones65[64:65, 0:64],
                                 rhs=recip[64:65, :], start=True, stop=True)
                bc_sb = sm_p.tile([64, 512], F32, tag="bc", name=f"bc{h}{tb}")
                nc.vector.tensor_copy(out=bc_sb, in_=ps_bc)
                nc.vector.tensor_tensor(out=attnT[tb][:, h, :],
                                        in0=ps_a[0:64, :], in1=bc_sb,
                                        op=ALU.mult)
                nc.sync.dma_start(out=a2a_io[h][0][tb], in_=attnT[tb][:, h, :])

            for h in range(HPC):
                for ta, tb2 in ((0, 1), (2, 3), (4, 5), (6, 7)):
                    ps_aA = ps_a_p.tile([65, 512], F32, tag="pa", name=f"paA{h}{ta}")
                    ps_aB = ps_a_p.tile([65, 512], F32, tag="pa", name=f"paB{h}{tb2}")
                    na, nb2 = 4 * (ta + 1), 4 * (tb2 + 1)
                    # scores run one step ahead of the AV consumers so the
                    # exp (ScalarE) latency never stalls the PE stream
                    prev_a = prev_b = None
                    for i in range(nb2 + 1):
                        w_a = at_scores(h, ta, i) if i < na else None
                        w_b = at_scores(h, tb2, i) if i < nb2 else None
                        if prev_a is not None:
                            at_av(h, ta, i - 1, ps_aA, prev_a)
                        if prev_b is not None:
                            at_av(h, tb2, i - 1, ps_aB, prev_b)
                        prev_a, prev_b = w_a, w_b
                    at_finish(h, ta, ps_aA)
                    at_finish(h, tb2, ps_aB)
                nc.gpsimd.collective_compute(
                    "AllToAll", ALU.bypass,
                    replica_groups=[list(range(N_CORES))],
                    ins=[a2a_io[h][0][:]], outs=[a2a_io[h][1][:]])

        attn_stack.close()

        # ---------- output projection + residual + LN2 + transpose ----------
        late = stack.enter_context(tc.tile_pool(name="late", bufs=1))
        x_rows = late.tile([128, 4, C], F32)
        for tt in range(4):
            nc.sync.dma_start(out=x_rows[:, tt, :],
                              in_=d["xr"][128 * tt:128 * tt + 128])
        x2_sb = late.tile([128, 4, C], F32)
        h2T = late.tile([128, NCT, 512], BF16)
        with (
            tc.tile_pool(name="pr_g", bufs=1) as g_p,
            tc.tile_pool(name="pr_wp", bufs=1) as wp_p,
            tc.tile_pool(name="pr_ps", bufs=4, space="PSUM") as pr_ps,
            tc.tile_pool(name="pr_tmp", bufs=4) as pr_tmp,
            tc.tile_pool(name="tr2_ps", bufs=4, space="PSUM") as tr2_ps,
        ):
            gat = g_p.tile([128, NCT, 512], BF16)
            for ci in range(NCT):
                nc.sync.dma_start(out=gat[0:64, ci, :], in_=d["a2ao0"][ci])
                nc.sync.dma_start(out=gat[64:128, ci, :], in_=d["a2ao1"][ci])
            wp_sb = wp_p.tile([128, NCT, C], BF16)
            for ci in range(NCT):
                nc.sync.dma_start(out=wp_sb[:, ci, :], in_=d["wp"][ci])

            for tt in range(4):
                for nb in range(2):
                    ns = slice(512 * nb, 512 * nb + 512)
                    ps_p = pr_ps.tile([128, 512], F32, tag="pp")
                    # x2 = x + bp + attn @ Wp accumulated fully in PSUM: the
                    # fp32 identity matmul injects the residual, the K=1 ones
                    # matmul injects the bias row.
                    nc.tensor.matmul(ps_p, lhsT=ident_f, rhs=x_rows[:, tt, ns],
                                     start=True, stop=False)
                    nc.tensor.matmul(ps_p, lhsT=ones1, rhs=bp_row[:, ns],
                                     start=False, stop=False)
                    for ci in range(NCT):
                        nc.tensor.matmul(
                            ps_p,
                            lhsT=gat[:, ci, 128 * tt:128 * tt + 128],
                            rhs=wp_sb[:, ci, ns],
                            start=False, stop=(ci == NCT - 1))
                    nc.vector.tensor_copy(out=x2_sb[:, tt, ns], in_=ps_p)
                # LN2 for this row-tile
                stats = pr_tmp.tile([128, 2, 6], F32, tag="stats")
                x2r = x2_sb[:, tt, :].rearrange("p (s d) -> p s d", s=2)
                for sg in range(2):
                    nc.vector.bn_stats(out=stats[:, sg, :], in_=x2r[:, sg, :])
                mv = pr_tmp.tile([128, 2], F32, tag="mv")
                nc.vector.bn_aggr(out=mv, in_=stats)
                rstd = pr_tmp.tile([128, 1], F32, tag="rstd")
                nc.scalar.activation(out=rstd, in_=mv[:, 1:2], func=AF.Sqrt,
                                     bias=eps_t, scale=1.0)
                nc.vector.reciprocal(out=rstd, in_=rstd)
                h2c = pr_tmp.tile([128, C], BF16, tag="h2c")
                nc.vector.tensor_scalar(out=h2c, in0=x2_sb[:, tt, :],
                                        scalar1=mv[:, 0:1], scalar2=rstd,
                                        op0=ALU.subtract, op1=ALU.mult)
                for ci in range(NCT):
                    ps_t = tr2_ps.tile([128, 128], BF16, tag="tr")
                    nc.tensor.transpose(ps_t, h2c[:, 128 * ci:128 * ci + 128], ident)
                    nc.vector.tensor_copy(
                        out=h2T[:, ci, 128 * tt:128 * tt + 128], in_=ps_t)

        # ---------- FFN ----------
        ff_sb = late.tile([128, NFB, 512], BF16)
        with (
            tc.tile_pool(name="f1_w", bufs=3) as f1_w,
            tc.tile_pool(name="f1_ps", bufs=4, space="PSUM") as f1_ps,
        ):
            for fb in range(NFB):
                w1_sb = f1_w.tile([128, NCT, 128], BF16, tag="w1")
                nc.sync.dma_start(out=w1_sb, in_=d["w1"][fb])
                ps_f = f1_ps.tile([128, 512], F32, tag="pf")
                for ci in range(NCT):
                    nc.tensor.matmul(ps_f, lhsT=w1_sb[:, ci, :], rhs=h2T[:, ci, :],
                                     start=(ci == 0), stop=(ci == NCT - 1))
                nc.scalar.activation(out=ff_sb[:, fb, :], in_=ps_f, func=AF.Relu,
                                     bias=b1_sb[:, fb:fb + 1], scale=1.0)

        with (
            tc.tile_pool(name="f2_w", bufs=3) as f2_w,
            tc.tile_pool(name="f2_ps", bufs=1, space="PSUM") as f2_ps,
            tc.tile_pool(name="out_sb", bufs=2) as out_p,
        ):
            ps_o = [[f2_ps.tile([128, 512], F32, tag=f"o{tt}{nb}", name=f"o{tt}{nb}")
                     for nb in range(2)] for tt in range(4)]
            for tt in range(4):
                for nb in range(2):
                    ns = slice(512 * nb, 512 * nb + 512)
                    nc.tensor.matmul(ps_o[tt][nb], lhsT=ident_f,
                                     rhs=x2_sb[:, tt, ns], start=True, stop=False)
                    nc.tensor.matmul(ps_o[tt][nb], lhsT=ones1,
                                     rhs=b2_row[:, ns], start=False, stop=False)
            for fb in range(NFB):
                w2_sb = f2_w.tile([128, C], BF16, tag="w2")
                nc.sync.dma_start(out=w2_sb, in_=d["w2"][fb])
                for tt in range(4):
                    for nb in range(2):
                        nc.tensor.matmul(
                            ps_o[tt][nb],
                            lhsT=ff_sb[:, fb, 128 * tt:128 * tt + 128],
                            rhs=w2_sb[:, 512 * nb:512 * nb + 512],
                            start=False, stop=(fb == NFB - 1))
            for tt in range(4):
                o_t = out_p.tile([128, C], F32, tag="o")
                for nb in range(2):
                    ns = slice(512 * nb, 512 * nb + 512)
                    nc.vector.tensor_copy(out=o_t[:, ns], in_=ps_o[tt][nb])
                nc.sync.dma_start(out=d["out"][128 * tt:128 * tt + 128], in_=o_t)


_NC_CACHE = None


def _get_program():
    global _NC_CACHE
    if _NC_CACHE is None:
        _NC_CACHE = build_program()
    return _NC_CACHE


def make_in_maps(inputs):
    x = _f32(inputs["x"]).reshape(T, C)
    Wq = _f32(inputs["Wq"])
    Wk = _f32(inputs["Wk"])
    Wv = _f32(inputs["Wv"])
    Wp = _f32(inputs["Wp"])
    bp = _f32(inputs["bp"])
    W1 = _f32(inputs["W1"])
    b1 = _f32(inputs["b1"])
    W2 = _f32(inputs["W2"])
    b2 = _f32(inputs["b2"])
    g1 = _f32(inputs["g1"])
    be1 = _f32(inputs["be1"])
    g2 = _f32(inputs["g2"])
    be2 = _f32(inputs["be2"])

    # fold LN affine params into adjacent projections
    Wq_f = g1[None, :, None] * Wq          # [H, C, hd]
    Wk_f = g1[None, :, None] * Wk
    Wv_f = g1[None, :, None] * Wv
    bq_f = np.einsum("c,hcd->hd", be1, Wq)  # [H, hd]
    bk_f = np.einsum("c,hcd->hd", be1, Wk)
    bv_f = np.einsum("c,hcd->hd", be1, Wv)
    W1_f = g2[:, None] * W1                 # [C, FF]
    b1_f = b1 + be2 @ W1                    # [FF]

    # causal masks for the 4 diagonal 128x512 sub-tiles
    tk_l = np.arange(128)[:, None]
    tq_l = np.arange(512)[None, :]
    cmask = np.stack([(tk_l + 128 * r <= tq_l) for r in range(4)]).astype(np.float32)

    w1_host = _bf16(W1_f.reshape(NCT, 128, NFB, 128).transpose(2, 1, 0, 3))
    b1_host = _f32(b1_f.reshape(NFB, 128).T)
    w2_host = _bf16(W2.reshape(NFB, 128, C))
    wp_host = _bf16(Wp.reshape(NCT, 128, C))
    cm_host = _bf16(cmask)
    x_host = _f32(x)
    bp_host = _f32(bp)
    b2_host = _f32(b2)

    in_maps = []
    for j in range(N_CORES):
        hsl = slice(HPC * j, HPC * j + HPC)
        wq_l = np.concatenate([Wq_f[h] for h in range(HPC * j, HPC * j + HPC)], axis=1)
        wk_l = np.concatenate([Wk_f[h] for h in range(HPC * j, HPC * j + HPC)], axis=1)
        wv_l = np.concatenate([Wv_f[h] for h in range(HPC * j, HPC * j + HPC)], axis=1)
        in_maps.append({
            "x": x_host,
            "xr": _f32(x[ROWS * j:ROWS * j + ROWS]),
            "wq": _bf16(wq_l.reshape(NCT, 128, D2)),
            "wk": _bf16(wk_l.reshape(NCT, 128, D2)),
            "wv": _bf16(wv_l.reshape(NCT, 128, D2)),
            "bq": _f32(bq_f[hsl].reshape(D2)),
            "bk": _f32(bk_f[hsl].reshape(D2)),
            "bv": _f32(bv_f[hsl].reshape(D2)),
            "wp": wp_host,
            "bp": bp_host,
            "w1": w1_host,
            "b1": b1_host,
            "w2": w2_host,
            "b2": b2_host,
            "cmask": cm_host,
        })
    return in_maps


def run(inputs, trace=False, trace_kwargs=None):
    nc = _get_program()
    in_maps = make_in_maps(inputs)
    res = run_bass_kernel_spmd(nc, in_maps, core_ids=list(range(N_CORES)),
                               trace=trace, **(trace_kwargs or {}))
    out = np.concatenate([res.results[j]["out"] for j in range(N_CORES)], axis=0)
    return out.reshape(1, T, C).astype(np.float32), res


def kernel(**inputs):
    out, _ = run(inputs)
    return out

